# revision 12
# baseline (speedup 1.0000x reference)
"""Trainium2 Bass kernel for nn_DMoN3P (tripartite DMoN modularity loss).

Strategy (8 NeuronCores, SPMD):
- Destination-sharded edges: core c owns Y rows [c*12500, (c+1)*12500). Each
  core segment-sums its A = sum_e w_e * softmax(Sx)[i_e] and C over its Y
  range entirely on-core (no [Y,K] all-reduce needed).
- Host prep (data movement only): sort/pad edges by (src chunk, dest block),
  build int16 gather indices and per-edge (dest-col, weight) arrays.
- Gather raw logits rows via dma_gather (4 SWDGE queues), exp on ACT, per-edge
  1/Z and w folded into a per-edge scale, segment-sum via one-hot matmul into
  PSUM, accumulated into SBUF A/C (with degree in a 65th column).
- Tiny [K,K] modularity math replicated per core after a 34KB AllReduce;
  second scalar AllReduce for Q_obs.
"""
import sys
from contextlib import ExitStack

sys.path.insert(0, "/opt/trn_rl_repo")

import numpy as np
import ml_dtypes

import concourse.bass as bass
import concourse.bacc as bacc
import concourse.mybir as mybir
import concourse.tile as tile
from concourse._compat import get_trn_type
from concourse.bass_utils import run_bass_kernel_spmd

F32 = mybir.dt.float32
BF16 = mybir.dt.bfloat16
I16 = mybir.dt.int16
AX = mybir.AxisListType.X
OP = mybir.AluOpType
AF = mybir.ActivationFunctionType

P = 128
NCORE = 8
TBATCH = 64          # tiles (of 128 edges) per gather batch
GRP = 7              # dest blocks per PSUM group (7*65=455 <= 512 fp32 bank)
NQUEUE = 4

BETA = 3.0
LAM = 1e-4
GAMMA_ = 1.0
ENTW = 1e-3
EPS = 1e-9


class _Geo:
    def __init__(self, num=100000, k=64, nchunk=4):
        self.NUM = num
        self.K = k
        self.NCHUNK = nchunk
        self.CHUNK = num // nchunk
        assert self.CHUNK <= 32768, "int16 gather index range"
        assert num % NCORE == 0 and num % nchunk == 0
        self.YC = num // NCORE
        self.NBLK = (self.YC + P - 1) // P


GEO = _Geo()

_TRACE = [False]


# ---------------------------------------------------------------- host prep
def _prep_edges(geo, edge_index, edge_weight):
    """Per-core edge streams: chunk-major, dest-block minor, padded to 128-edge
    tiles with identical tile counts across cores (SPMD uniformity).

    Y mapping is p-major: j_local = p*NBLK + b  (p in [0,128), b in [0,NBLK)).
    """
    i_all = np.asarray(edge_index[0], np.int64)
    j_all = np.asarray(edge_index[1], np.int64)
    w_all = np.asarray(edge_weight, np.float32)
    NBLK, CHUNK, YC = geo.NBLK, geo.CHUNK, geo.YC

    cores = []
    counts = np.zeros((NCORE, geo.NCHUNK, NBLK), np.int64)
    for c in range(NCORE):
        sel = (j_all // YC) == c
        ic = i_all[sel]
        jl = j_all[sel] - c * YC
        wc = w_all[sel]
        chunk = ic // CHUNK
        il = (ic % CHUNK).astype(np.int64)
        pcol = jl // NBLK
        blk = jl % NBLK
        order = np.lexsort((blk, chunk))
        cores.append((il[order], pcol[order], wc[order], chunk[order], blk[order]))
        np.add.at(counts[c], (chunk[order], blk[order]), 1)

    ntiles = np.maximum(1, -(-counts.max(axis=0) // P))  # [NCHUNK, NBLK]
    NT = int(ntiles.sum())

    ncell = geo.NCHUNK * NBLK
    cell_t0 = np.concatenate(([0], np.cumsum(ntiles.reshape(-1))))  # tile offset
    per_core = []
    for c in range(NCORE):
        il, pcol, wc, chunk, blk = cores[c]
        key = chunk * NBLK + blk
        bounds = np.searchsorted(key, np.arange(ncell + 1))
        # destination slot for each edge: cell tile base * P + rank within cell
        ranks = np.arange(len(il)) - bounds[key]
        slots = cell_t0[key] * P + ranks
        idx16 = np.zeros(NT * P, np.int16)
        jcol = np.zeros(NT * P, np.float32)
        wpad = np.zeros(NT * P, np.float32)
        idx16[slots] = il
        jcol[slots] = pcol
        wpad[slots] = wc
        per_core.append((idx16.reshape(NT, P), jcol.reshape(NT, P),
                         wpad.reshape(NT, P)))

    batches = []           # (chunk, t0, T)
    t0 = 0
    for ch in range(geo.NCHUNK):
        tc_ = int(ntiles[ch].sum())
        off = 0
        while off < tc_:
            T = min(TBATCH, tc_ - off)
            batches.append((ch, t0 + off, T))
            off += T
        t0 += tc_

    NB = len(batches)
    out_cores = []
    for c in range(NCORE):
        idx16, jcol, wpad = per_core[c]
        idxw = np.zeros((NB, P, TBATCH * P // 16), np.int16)
        for bi, (ch, t0_, T) in enumerate(batches):
            flat = idx16[t0_:t0_ + T].reshape(-1)
            wr = flat.reshape(-1, 16).T                    # [16, T*8]
            idxw[bi, :, :wr.shape[1]] = np.tile(wr, (8, 1))
        jb = np.ascontiguousarray(jcol.T).astype(ml_dtypes.bfloat16)
        wf = np.ascontiguousarray(wpad.T).astype(np.float32)
        out_cores.append({"idx": idxw, "jb": jb, "wf": wf})

    tile_blk = np.zeros(NT, np.int64)
    first = np.zeros(NT, bool)
    last = np.zeros(NT, bool)
    t = 0
    for ch in range(geo.NCHUNK):
        for b in range(NBLK):
            nt = int(ntiles[ch, b])
            tile_blk[t:t + nt] = b
            first[t] = True
            last[t + nt - 1] = True
            t += nt
    meta = {"ntiles": ntiles, "NT": NT, "batches": batches,
            "tile_blk": tile_blk, "first": first, "last": last}
    return meta, out_cores


# ---------------------------------------------------------------- builder
def _build(geo, meta_xy, meta_yz):
    NBLK, K, YC = geo.NBLK, geo.K, geo.YC
    KB = NBLK * K
    nc = bacc.Bacc(get_trn_type() or "TRN2", target_bir_lowering=False,
                   debug=False, num_swdge_queues=NQUEUE)
    # activation() float bias/scale values must exist as const APs
    for v in (EPS, BETA):
        t = nc.alloc_sbuf_tensor(f"const-float32-{v}", [P, 1], F32)
        nc.gpsimd.memset(t.ap(), v)
        nc.const_aps.aps[(F32, v)] = t.ap()
    nc.all_engine_barrier()

    sxl = nc.dram_tensor("sxl", [geo.NUM, K], F32, kind="ExternalInput")
    szl = nc.dram_tensor("szl", [geo.NUM, K], F32, kind="ExternalInput")
    sxs = nc.dram_tensor("sxs", [YC, K], F32, kind="ExternalInput")
    sys_ = nc.dram_tensor("sys", [YC, K], F32, kind="ExternalInput")
    szs = nc.dram_tensor("szs", [YC, K], F32, kind="ExternalInput")
    ident_in = nc.dram_tensor("ident", [P, P], F32, kind="ExternalInput")
    iota_in = nc.dram_tensor("iota", [P, P], BF16, kind="ExternalInput")
    ones_in = nc.dram_tensor("ones", [P, P + 1], F32, kind="ExternalInput")
    vmask_in = nc.dram_tensor("vmask", [P, NBLK], F32, kind="ExternalInput")

    ins = {}
    for s, meta in (("xy", meta_xy), ("yz", meta_yz)):
        NB = len(meta["batches"])
        NT = meta["NT"]
        ins[s] = {
            "idx": nc.dram_tensor(f"idx_{s}", [NB, P, TBATCH * 8], I16,
                                  kind="ExternalInput"),
            "jb": nc.dram_tensor(f"jb_{s}", [P, NT], BF16, kind="ExternalInput"),
            "wf": nc.dram_tensor(f"wf_{s}", [P, NT], F32, kind="ExternalInput"),
        }
    loss_out = nc.dram_tensor("loss", [1, 1], F32, kind="ExternalOutput")

    ARS = 2 * K * K + 256
    ar1_in = nc.dram_tensor("ar1_in", [ARS], F32)
    ar1_out = nc.dram_tensor("ar1_out", [ARS], F32, addr_space="Shared")
    ar2_in = nc.dram_tensor("ar2_in", [64], F32)
    ar2_out = nc.dram_tensor("ar2_out", [64], F32, addr_space="Shared")
    groups = [list(range(NCORE))]

    with tile.TileContext(nc) as tc, ExitStack() as es:
        pp = es.enter_context(tc.tile_pool(name="persist", bufs=1))
        sp = es.enter_context(tc.tile_pool(name="small", bufs=1))
        stp = es.enter_context(tc.tile_pool(name="statps", bufs=1, space="PSUM"))

        ident = pp.tile([P, P], F32)
        nc.sync.dma_start(ident[:], ident_in[:])
        iota = pp.tile([P, P], BF16)
        nc.sync.dma_start(iota[:], iota_in[:])
        ones = pp.tile([P, P + 1], F32)
        nc.sync.dma_start(ones[:], ones_in[:])
        vmask = pp.tile([P, NBLK], F32)
        nc.sync.dma_start(vmask[:], vmask_in[:])

        A_sb = pp.tile([P, NBLK * (K + 1)], F32)
        C_sb = pp.tile([P, NBLK * (K + 1)], F32)
        SY_sb = pp.tile([P, KB], F32)

        stats = stp.tile([1, 256], F32)

        # ---------------- phase 0: shard softmax + stats --------------------
        with tc.tile_pool(name="shard", bufs=1) as shp:
            def shard_stats(src, col_off, ent_col, sy_dst):
                sh = shp.tile([P, KB], F32, tag="shard")
                full_p = YC // NBLK
                nfull = full_p * NBLK
                rem = YC - nfull
                if rem or full_p + 1 < P:
                    nc.vector.memset(sh[:], 0.0)
                nc.sync.dma_start(
                    sh[:full_p, :],
                    src[0:nfull].rearrange("(p b) k -> p (b k)", b=NBLK))
                if rem:
                    nc.sync.dma_start(
                        sh[full_p:full_p + 1, 0:rem * K],
                        src[nfull:YC].rearrange("r k -> (r k)")[None, :])
                nc.scalar.activation(out=sh[:], in_=sh[:], func=AF.Exp)
                z = sp.tile([P, NBLK], F32, tag="z0")
                nc.vector.reduce_sum(
                    z[:], sh[:].rearrange("p (b k) -> p b k", k=K), axis=AX)
                nc.vector.reciprocal(z[:], z[:])
                # zero out invalid (p, b) cells via the validity mask
                nc.vector.tensor_tensor(out=z[:], in0=z[:], in1=vmask[:],
                                        op=OP.mult)
                dst = sy_dst if sy_dst is not None else sh
                nc.vector.tensor_tensor(
                    out=dst[:].rearrange("p (b k) -> p b k", k=K),
                    in0=sh[:].rearrange("p (b k) -> p b k", k=K),
                    in1=z[:, :, None].to_broadcast([P, NBLK, K]),
                    op=OP.mult)
                t1 = sp.tile([P, K], F32, tag="t1c")
                nc.vector.reduce_sum(
                    t1[:], dst[:].rearrange("p (b k) -> p k b", k=K), axis=AX)
                nc.tensor.matmul(out=stats[:, col_off:col_off + K],
                                 lhsT=ones[:, 0:1], rhs=t1[:],
                                 start=True, stop=True)
                ln = shp.tile([P, KB], F32, tag="lnsh")
                nc.scalar.activation(out=ln[:], in_=dst[:], func=AF.Ln, bias=EPS)
                nc.vector.tensor_tensor(out=ln[:], in0=ln[:], in1=dst[:],
                                        op=OP.mult)
                er = sp.tile([P, 1], F32, tag="entr")
                nc.vector.reduce_sum(er[:], ln[:], axis=AX)
                nc.tensor.matmul(out=stats[:, ent_col:ent_col + 1],
                                 lhsT=ones[:, 0:1], rhs=er[:],
                                 start=True, stop=True)

            shard_stats(sxs, 0, 192, None)
            shard_stats(sys_, 64, 193, SY_sb)
            shard_stats(szs, 128, 194, None)

        # ---------------- phase 1: edge segment sums ------------------------
        with (
            tc.tile_pool(name="gath", bufs=2) as gp,
            tc.tile_pool(name="oneh", bufs=2) as op_,
            tc.tile_pool(name="gext", bufs=2) as gxp,
            tc.tile_pool(name="idxp", bufs=3) as ixp,
            tc.tile_pool(name="perb", bufs=3) as pbp,
            tc.tile_pool(name="setc", bufs=1) as scp,
            tc.tile_pool(name="apsum", bufs=2, space="PSUM") as apsp,
        ):
            def process_set(tab, io, meta, acc):
                NT = meta["NT"]
                jb = scp.tile([P, NT], BF16, tag="jbt")
                nc.sync.dma_start(jb[:], io["jb"][:])
                wf = scp.tile([P, NT], F32, tag="wft")
                nc.sync.dma_start(wf[:], io["wf"][:])
                tile_blk = meta["tile_blk"]
                first, last = meta["first"], meta["last"]
                qrr = [0]
                cur = {"psg": None, "grp": -1, "ch": -1}

                def close_grp():
                    g, ch = cur["grp"], cur["ch"]
                    lo = g * GRP * (K + 1)
                    width = min(GRP, NBLK - g * GRP) * (K + 1)
                    if ch == 0:
                        nc.vector.tensor_copy(
                            out=acc[:, lo:lo + width],
                            in_=cur["psg"][:, 0:width])
                    else:
                        nc.vector.tensor_tensor(
                            out=acc[:, lo:lo + width],
                            in0=acc[:, lo:lo + width],
                            in1=cur["psg"][:, 0:width], op=OP.add)
                    cur["psg"] = None

                for bi, (ch, t0, T) in enumerate(meta["batches"]):
                    it = ixp.tile([P, TBATCH * 8], I16, tag="idxt")
                    nc.sync.dma_start(it[:], io["idx"][bi])
                    gt = gp.tile([P, TBATCH * K], F32, tag="gt")
                    nidx = T * P
                    nc.gpsimd.dma_gather(
                        gt[:].rearrange("p (t k) -> p t k", k=K)[:, 0:T, :],
                        tab[ch * geo.CHUNK:(ch + 1) * geo.CHUNK, :],
                        it[:, 0:nidx // 16], nidx, nidx, K,
                        single_packet=False, queue_num=qrr[0])
                    qrr[0] = (qrr[0] + 1) % NQUEUE
                    nc.scalar.activation(out=gt[:, 0:T * K], in_=gt[:, 0:T * K],
                                         func=AF.Exp)
                    z = pbp.tile([P, TBATCH], F32, tag="zb")
                    nc.vector.reduce_sum(
                        z[:, 0:T],
                        gt[:].rearrange("p (t k) -> p t k", k=K)[:, 0:T, :],
                        axis=AX)
                    nc.vector.reciprocal(z[:, 0:T], z[:, 0:T])
                    ct = pbp.tile([P, TBATCH], F32, tag="cb")
                    nc.vector.tensor_tensor(out=ct[:, 0:T], in0=z[:, 0:T],
                                            in1=wf[:, t0:t0 + T], op=OP.mult)
                    gx = gxp.tile([P, TBATCH * (K + 1)], BF16, tag="gx")
                    gxv = gx[:].rearrange("p (t k) -> p t k", k=K + 1)
                    nc.vector.tensor_tensor(
                        out=gxv[:, 0:T, 0:K],
                        in0=gt[:].rearrange("p (t k) -> p t k", k=K)[:, 0:T, :],
                        in1=ct[:, 0:T, None].to_broadcast([P, T, K]),
                        op=OP.mult)
                    nc.vector.tensor_copy(out=gxv[:, 0:T, K:K + 1],
                                          in_=wf[:, t0:t0 + T, None])
                    ob = op_.tile([P, TBATCH * P], BF16, tag="ob")
                    nc.vector.tensor_tensor(
                        out=ob[:].rearrange("p (t q) -> p t q", q=P)[:, 0:T, :],
                        in0=iota[:, None, :].to_broadcast([P, T, P]),
                        in1=jb[:, t0:t0 + T, None].to_broadcast([P, T, P]),
                        op=OP.is_equal)
                    for t in range(T):
                        tg = t0 + t
                        b = int(tile_blk[tg])
                        g = b // GRP
                        if g != cur["grp"] or ch != cur["ch"]:
                            if cur["psg"] is not None:
                                close_grp()
                            cur["psg"] = apsp.tile([P, GRP * (K + 1)], F32,
                                                   tag="apsg", name="apsg")
                            cur["grp"], cur["ch"] = g, ch
                        lo = (b % GRP) * (K + 1)
                        nc.tensor.matmul(
                            out=cur["psg"][:, lo:lo + K + 1],
                            lhsT=ob[:, t * P:(t + 1) * P],
                            rhs=gx[:, t * (K + 1):(t + 1) * (K + 1)],
                            start=bool(first[tg]), stop=bool(last[tg]))
                if cur["psg"] is not None:
                    close_grp()

            process_set(sxl, ins["xy"], meta_xy, A_sb)
            process_set(szl, ins["yz"], meta_yz, C_sb)

        # ---------------- phase 2a: omega, Mnorm, E partials ----------------
        dX = A_sb[:].rearrange("p (b k) -> p b k", k=K + 1)[:, :, K]
        dZ = C_sb[:].rearrange("p (b k) -> p b k", k=K + 1)[:, :, K]
        prod = sp.tile([P, NBLK], F32, tag="prod")
        nc.vector.tensor_tensor(out=prod[:], in0=dX, in1=dZ, op=OP.mult)
        valid = sp.tile([P, NBLK], F32, tag="valid")
        nc.vector.tensor_scalar(out=valid[:], in0=prod[:], scalar1=0.0,
                                scalar2=None, op0=OP.not_equal)
        omega = sp.tile([P, NBLK], F32, tag="omega")
        nc.vector.tensor_scalar(out=omega[:], in0=prod[:], scalar1=EPS,
                                scalar2=None, op0=OP.add)
        nc.vector.reciprocal(omega[:], omega[:])
        nc.vector.tensor_tensor(out=omega[:], in0=omega[:], in1=valid[:],
                                op=OP.mult)
        mn = sp.tile([P, NBLK], F32, tag="mn")
        nc.vector.tensor_tensor(out=mn[:], in0=prod[:], in1=valid[:], op=OP.mult)
        mnr = sp.tile([P, 1], F32, tag="mnr")
        nc.vector.reduce_sum(mnr[:], mn[:], axis=AX)
        nc.tensor.matmul(out=stats[:, 195:196], lhsT=ones[:, 0:1],
                         rhs=mnr[:], start=True, stop=True)

        bp = es.enter_context(tc.tile_pool(name="blk", bufs=4))
        with tc.tile_pool(name="epsum", bufs=1, space="PSUM") as eps_:
            exy_ps = eps_.tile([K, K], F32, tag="exy")
            eyz_ps = eps_.tile([K, K], F32, tag="eyz")
            for b in range(NBLK):
                ay = bp.tile([P, K], F32, tag="ay")
                nc.vector.tensor_scalar(
                    out=ay[:], in0=A_sb[:, b * (K + 1):b * (K + 1) + K],
                    scalar1=omega[:, b:b + 1], scalar2=None, op0=OP.mult)
                cy = bp.tile([P, K], F32, tag="cy")
                nc.vector.tensor_scalar(
                    out=cy[:], in0=C_sb[:, b * (K + 1):b * (K + 1) + K],
                    scalar1=omega[:, b:b + 1], scalar2=None, op0=OP.mult)
                nc.tensor.matmul(out=exy_ps[:], lhsT=ay[:],
                                 rhs=SY_sb[:, b * K:(b + 1) * K],
                                 start=(b == 0), stop=(b == NBLK - 1))
                nc.tensor.matmul(out=eyz_ps[:], lhsT=SY_sb[:, b * K:(b + 1) * K],
                                 rhs=cy[:], start=(b == 0), stop=(b == NBLK - 1))
            exy_sb = sp.tile([K, K], F32, tag="exysb")
            nc.vector.tensor_copy(out=exy_sb[:], in_=exy_ps[:])
            eyz_sb = sp.tile([K, K], F32, tag="eyzsb")
            nc.vector.tensor_copy(out=eyz_sb[:], in_=eyz_ps[:])
        stats_sb = sp.tile([1, 256], F32, tag="statsb")
        nc.vector.memset(stats_sb[:], 0.0)
        nc.vector.tensor_copy(out=stats_sb[:, 0:196], in_=stats[:, 0:196])

        nc.sync.dma_start(ar1_in[0:K * K], exy_sb[:])
        nc.sync.dma_start(ar1_in[K * K:2 * K * K], eyz_sb[:])
        nc.sync.dma_start(ar1_in[2 * K * K:ARS], stats_sb[:])
        nc.gpsimd.collective_compute(
            "AllReduce", OP.add, replica_groups=groups,
            ins=[ar1_in[:]], outs=[ar1_out[:]])
        exy = sp.tile([K, K], F32, tag="exyg")
        nc.sync.dma_start(exy[:], ar1_out[0:K * K].rearrange("(a b) -> a b", b=K))
        eyz = sp.tile([K, K], F32, tag="eyzg")
        nc.sync.dma_start(eyz[:],
                          ar1_out[K * K:2 * K * K].rearrange("(a b) -> a b", b=K))
        stg = sp.tile([1, 256], F32, tag="stg")
        nc.sync.dma_start(stg[:], ar1_out[2 * K * K:ARS][None, :])

        # ---------------- phase 2b: alpha/gamma + Q_exp (replicated) --------
        with tc.tile_pool(name="p2psum", bufs=2, space="PSUM") as ps2:
            def pe_t(src, rows, cols, tag):
                pt = ps2.tile([P, P], F32, tag="scr2b", name="pt")
                nc.tensor.transpose(out=pt[0:cols, 0:rows], in_=src,
                                    identity=ident[0:rows, 0:rows])
                out = sp.tile([cols, rows], F32, tag=tag)
                nc.vector.tensor_copy(out=out[:], in_=pt[0:cols, 0:rows])
                return out

            invmn = sp.tile([P, 1], F32, tag="invmn")
            bps = ps2.tile([P, 1], F32, tag="scr2b", name="bps")
            nc.tensor.matmul(out=bps[:], lhsT=ones[0:1, 1:P + 1],
                             rhs=stg[:, 195:196], start=True, stop=True)
            nc.vector.tensor_scalar(out=invmn[:], in0=bps[:], scalar1=EPS,
                                    scalar2=None, op0=OP.add)
            nc.vector.reciprocal(invmn[:], invmn[:])

            exn = sp.tile([K, K], F32, tag="exn")
            nc.vector.tensor_scalar(out=exn[:], in0=exy[:],
                                    scalar1=invmn[0:K, :], scalar2=None,
                                    op0=OP.mult)
            ezn = sp.tile([K, K], F32, tag="ezn")
            nc.vector.tensor_scalar(out=ezn[:], in0=eyz[:],
                                    scalar1=invmn[0:K, :], scalar2=None,
                                    op0=OP.mult)

            def softmax_rows(src, tag):
                e = sp.tile([K, K], F32, tag=tag + "e")
                nc.scalar.activation(out=e[:], in_=src[:], func=AF.Exp,
                                     scale=BETA)
                zz = sp.tile([K, 1], F32, tag=tag + "z")
                nc.vector.reduce_sum(zz[:], e[:], axis=AX)
                nc.vector.reciprocal(zz[:], zz[:])
                nc.vector.tensor_scalar(out=e[:], in0=e[:], scalar1=zz[:],
                                        scalar2=None, op0=OP.mult)
                return e

            exnT = pe_t(exn[:], K, K, "exnT")
            alphaT = softmax_rows(exnT, "aT")          # [m, l]
            alpha = pe_t(alphaT[:], K, K, "alpha")     # [l, m]
            gamma = softmax_rows(ezn, "gm")            # [m, n]
            gammaT = pe_t(gamma[:], K, K, "gmT")       # [n, m]

            aX = sp.tile([K, 1], F32, tag="aX")
            nc.vector.reduce_sum(aX[:], exn[:], axis=AX)
            aY = sp.tile([K, 1], F32, tag="aY")
            nc.vector.reduce_sum(aY[:], exnT[:], axis=AX)
            eznT = pe_t(ezn[:], K, K, "eznT")
            aZ = sp.tile([K, 1], F32, tag="aZ")
            nc.vector.reduce_sum(aZ[:], eznT[:], axis=AX)

            tl_ps = ps2.tile([K, 1], F32, tag="scr2b", name="tl_ps")
            nc.tensor.matmul(out=tl_ps[:], lhsT=alpha[:], rhs=aX[:],
                             start=True, stop=True)
            tr_ps = ps2.tile([K, 1], F32, tag="scr2b", name="tr_ps")
            nc.tensor.matmul(out=tr_ps[:], lhsT=gammaT[:], rhs=aZ[:],
                             start=True, stop=True)
            qe = sp.tile([K, 1], F32, tag="qe")
            nc.vector.tensor_tensor(out=qe[:], in0=tl_ps[:], in1=aY[:],
                                    op=OP.mult)
            nc.vector.tensor_tensor(out=qe[:], in0=qe[:], in1=tr_ps[:],
                                    op=OP.mult)
            qexp_ps = ps2.tile([1, 1], F32, tag="scr2b", name="qexp_ps")
            nc.tensor.matmul(out=qexp_ps[:], lhsT=ones[0:K, 0:1], rhs=qe[:],
                             start=True, stop=True)
            qexp = sp.tile([1, 1], F32, tag="qexp")
            nc.vector.tensor_copy(out=qexp[:], in_=qexp_ps[:])

            # ---------------- phase 2c: Q_obs partial -----------------------
            qacc = sp.tile([P, K], F32, tag="qacc")
            nc.vector.memset(qacc[:], 0.0)
            for b in range(NBLK):
                ay = bp.tile([P, K], F32, tag="ay2")
                nc.vector.tensor_scalar(
                    out=ay[:], in0=A_sb[:, b * (K + 1):b * (K + 1) + K],
                    scalar1=omega[:, b:b + 1], scalar2=None, op0=OP.mult)
                cy = bp.tile([P, K], F32, tag="cy2")
                nc.vector.tensor_scalar(
                    out=cy[:], in0=C_sb[:, b * (K + 1):b * (K + 1) + K],
                    scalar1=omega[:, b:b + 1], scalar2=None, op0=OP.mult)
                ayT_ps = ps2.tile([K, P], F32, tag="ayT", bufs=1)
                nc.tensor.transpose(out=ayT_ps[:], in_=ay[:], identity=ident[:])
                ayT = bp.tile([K, P], F32, tag="ayTs")
                nc.vector.tensor_copy(out=ayT[:], in_=ayT_ps[:])
                cyT_ps = ps2.tile([K, P], F32, tag="cyT", bufs=1)
                nc.tensor.transpose(out=cyT_ps[:], in_=cy[:], identity=ident[:])
                cyT = bp.tile([K, P], F32, tag="cyTs")
                nc.vector.tensor_copy(out=cyT[:], in_=cyT_ps[:])
                l_ps = ps2.tile([P, K], F32, tag="lps", bufs=1)
                nc.tensor.matmul(out=l_ps[:], lhsT=ayT[:], rhs=alpha[:],
                                 start=True, stop=True)
                r_ps = ps2.tile([P, K], F32, tag="rps", bufs=1)
                nc.tensor.matmul(out=r_ps[:], lhsT=cyT[:], rhs=gammaT[:],
                                 start=True, stop=True)
                t1 = bp.tile([P, K], F32, tag="t1q")
                nc.vector.tensor_tensor(out=t1[:], in0=l_ps[:],
                                        in1=SY_sb[:, b * K:(b + 1) * K],
                                        op=OP.mult)
                nc.vector.tensor_tensor(out=t1[:], in0=t1[:], in1=r_ps[:],
                                        op=OP.mult)
                nc.vector.tensor_tensor(out=qacc[:], in0=qacc[:], in1=t1[:],
                                        op=OP.add)
            qr = sp.tile([P, 1], F32, tag="qr")
            nc.vector.reduce_sum(qr[:], qacc[:], axis=AX)
            qo_ps = ps2.tile([1, 1], F32, tag="scr2b", name="qo_ps")
            nc.tensor.matmul(out=qo_ps[:], lhsT=ones[:, 0:1], rhs=qr[:],
                             start=True, stop=True)
            qob = sp.tile([1, 64], F32, tag="qob")
            nc.vector.memset(qob[:], 0.0)
            nc.vector.tensor_copy(out=qob[:, 0:1], in_=qo_ps[:])
            nc.sync.dma_start(ar2_in[:], qob[:])
            nc.gpsimd.collective_compute(
                "AllReduce", OP.add, replica_groups=groups,
                ins=[ar2_in[:]], outs=[ar2_out[:]])
            qot = sp.tile([1, 1], F32, tag="qot")
            nc.sync.dma_start(qot[:], ar2_out[0:1][None, :])

            # ---------------- loss assembly ---------------------------------
            sc = sp.tile([1, 8], F32, tag="scr")
            nc.vector.tensor_scalar(out=sc[:, 0:1], in0=qot[:],
                                    scalar1=invmn[0:1, :], scalar2=None,
                                    op0=OP.mult)
            nc.vector.tensor_scalar(out=sc[:, 1:2], in0=qexp[:],
                                    scalar1=invmn[0:1, :], scalar2=None,
                                    op0=OP.mult)
            nc.vector.tensor_scalar(out=sc[:, 1:2], in0=sc[:, 1:2],
                                    scalar1=GAMMA_, scalar2=None, op0=OP.mult)
            nc.vector.tensor_tensor(out=sc[:, 2:3], in0=sc[:, 1:2],
                                    in1=sc[:, 0:1], op=OP.subtract)
            for mi in range(3):
                cs = stg[:, 64 * mi:64 * mi + K]
                sq = sp.tile([1, K], F32, tag="sqc")
                nc.vector.tensor_tensor(out=sq[:], in0=cs, in1=cs, op=OP.mult)
                fr = sp.tile([1, 1], F32, tag="frc")
                nc.vector.reduce_sum(fr[:], sq[:], axis=AX)
                nc.scalar.activation(out=fr[:], in_=fr[:], func=AF.Sqrt)
                nc.vector.tensor_scalar(
                    out=fr[:], in0=fr[:],
                    scalar1=float(LAM * np.sqrt(float(K)) / geo.NUM),
                    scalar2=float(-LAM), op0=OP.mult, op1=OP.add)
                nc.vector.tensor_tensor(out=sc[:, 2:3], in0=sc[:, 2:3],
                                        in1=fr[:], op=OP.add)
            er = sp.tile([1, 1], F32, tag="ereg")
            nc.vector.reduce_sum(er[:], stg[:, 192:195], axis=AX)
            nc.vector.tensor_scalar(out=er[:], in0=er[:],
                                    scalar1=float(ENTW / geo.NUM), scalar2=None,
                                    op0=OP.mult)
            nc.vector.tensor_tensor(out=sc[:, 2:3], in0=sc[:, 2:3],
                                    in1=er[:], op=OP.add)
            nc.sync.dma_start(loss_out[:], sc[:, 2:3])
    nc.compile()
    return nc


# ---------------------------------------------------------------- entry
def _make_in_maps(geo, Sx, Sy, Sz, cores_xy, cores_yz):
    ident = np.eye(P, dtype=np.float32)
    iota = np.tile(np.arange(P, dtype=np.float32), (P, 1)).astype(ml_dtypes.bfloat16)
    ones = np.ones((P, P + 1), np.float32)
    pb = np.arange(P)[:, None] * geo.NBLK + np.arange(geo.NBLK)[None, :]
    vmask_np = (pb < geo.YC).astype(np.float32)
    YC = geo.YC
    in_maps = []
    for c in range(NCORE):
        in_maps.append({
            "sxl": Sx, "szl": Sz,
            "sxs": np.ascontiguousarray(Sx[c * YC:(c + 1) * YC]),
            "sys": np.ascontiguousarray(Sy[c * YC:(c + 1) * YC]),
            "szs": np.ascontiguousarray(Sz[c * YC:(c + 1) * YC]),
            "ident": ident, "iota": iota, "ones": ones, "vmask": vmask_np,
            "idx_xy": cores_xy[c]["idx"], "jb_xy": cores_xy[c]["jb"],
            "wf_xy": cores_xy[c]["wf"],
            "idx_yz": cores_yz[c]["idx"], "jb_yz": cores_yz[c]["jb"],
            "wf_yz": cores_yz[c]["wf"],
        })
    return in_maps


def kernel(Sx_logits, Sy_logits, Sz_logits, edge_index_XY, edge_weight_XY,
           edge_index_YZ, edge_weight_YZ, _geo=None, _sim=False):
    geo = _geo or GEO
    Sx = np.ascontiguousarray(np.asarray(Sx_logits, np.float32))
    Sy = np.ascontiguousarray(np.asarray(Sy_logits, np.float32))
    Sz = np.ascontiguousarray(np.asarray(Sz_logits, np.float32))

    meta_xy, cores_xy = _prep_edges(geo, np.asarray(edge_index_XY),
                                    np.asarray(edge_weight_XY))
    meta_yz, cores_yz = _prep_edges(geo, np.asarray(edge_index_YZ),
                                    np.asarray(edge_weight_YZ))
    nc = _build(geo, meta_xy, meta_yz)
    in_maps = _make_in_maps(geo, Sx, Sy, Sz, cores_xy, cores_yz)

    if _sim:
        import concourse.bass_interp as bass_interp
        sim = bass_interp.MultiCoreSim(nc, NCORE)
        for c in range(NCORE):
            for k, v in in_maps[c].items():
                sim.cores[c].tensor(k)[:] = np.asarray(v)
        sim.simulate()
        return np.float32(np.asarray(sim.cores[0].tensor("loss"))[0, 0])

    res = run_bass_kernel_spmd(nc, in_maps, list(range(NCORE)),
                               trace=_TRACE[0])
    if _TRACE[0]:
        kernel.last_exec_ns = res.exec_time_ns
    return np.float32(res.results[0]["loss"][0, 0])


# revision 13
# speedup vs baseline: 1.5241x; 1.5241x over previous
"""Trainium2 Bass kernel for nn_DMoN3P (tripartite DMoN modularity loss).

Strategy (8 NeuronCores, SPMD):
- Destination-sharded edges: core c owns Y rows [c*12500, (c+1)*12500). Each
  core segment-sums its A = sum_e w_e * softmax(Sx)[i_e] and C over its Y
  range entirely on-core (no [Y,K] all-reduce needed).
- Host prep (data movement only): sort/pad edges by (src chunk, dest block),
  build int16 gather indices and per-edge (dest-col, weight) arrays.
- Gather raw logits rows via dma_gather (4 SWDGE queues), exp on ACT, per-edge
  1/Z and w folded into a per-edge scale, segment-sum via one-hot matmul into
  PSUM, accumulated into SBUF A/C (with degree in a 65th column).
- Tiny [K,K] modularity math replicated per core after a 34KB AllReduce;
  second scalar AllReduce for Q_obs.
"""
import sys
from contextlib import ExitStack

sys.path.insert(0, "/opt/trn_rl_repo")

import numpy as np
import ml_dtypes

import concourse.bass as bass
import concourse.bacc as bacc
import concourse.mybir as mybir
import concourse.tile as tile
from concourse._compat import get_trn_type
from concourse.bass_utils import run_bass_kernel_spmd

F32 = mybir.dt.float32
BF16 = mybir.dt.bfloat16
I16 = mybir.dt.int16
AX = mybir.AxisListType.X
OP = mybir.AluOpType
AF = mybir.ActivationFunctionType

P = 128
NCORE = 8
TBATCH = 64          # tiles (of 128 edges) per gather batch
GRP = 7              # dest blocks per PSUM group (7*65=455 <= 512 fp32 bank)
NQUEUE = 4

BETA = 3.0
LAM = 1e-4
GAMMA_ = 1.0
ENTW = 1e-3
EPS = 1e-9


class _Geo:
    def __init__(self, num=100000, k=64, nchunk=4):
        self.NUM = num
        self.K = k
        self.NCHUNK = nchunk
        self.CHUNK = num // nchunk
        assert self.CHUNK <= 32768, "int16 gather index range"
        assert num % NCORE == 0 and num % nchunk == 0
        self.YC = num // NCORE
        self.NBLK = (self.YC + P - 1) // P


GEO = _Geo()

_TRACE = [False]


# ---------------------------------------------------------------- host prep
def _prep_edges(geo, edge_index, edge_weight):
    """Per-core edge streams: chunk-major, dest-block minor, padded to 128-edge
    tiles with identical tile counts across cores (SPMD uniformity).

    Y mapping is p-major: j_local = p*NBLK + b  (p in [0,128), b in [0,NBLK)).
    """
    i_all = np.asarray(edge_index[0], np.int64)
    j_all = np.asarray(edge_index[1], np.int64)
    w_all = np.asarray(edge_weight, np.float32)
    NBLK, CHUNK, YC = geo.NBLK, geo.CHUNK, geo.YC

    cores = []
    counts = np.zeros((NCORE, geo.NCHUNK, NBLK), np.int64)
    for c in range(NCORE):
        sel = (j_all // YC) == c
        ic = i_all[sel]
        jl = j_all[sel] - c * YC
        wc = w_all[sel]
        chunk = ic // CHUNK
        il = (ic % CHUNK).astype(np.int64)
        pcol = jl // NBLK
        blk = jl % NBLK
        order = np.lexsort((blk, chunk))
        cores.append((il[order], pcol[order], wc[order], chunk[order], blk[order]))
        np.add.at(counts[c], (chunk[order], blk[order]), 1)

    ntiles = np.maximum(1, -(-counts.max(axis=0) // P))  # [NCHUNK, NBLK]
    NT = int(ntiles.sum())

    ncell = geo.NCHUNK * NBLK
    cell_t0 = np.concatenate(([0], np.cumsum(ntiles.reshape(-1))))  # tile offset
    per_core = []
    for c in range(NCORE):
        il, pcol, wc, chunk, blk = cores[c]
        key = chunk * NBLK + blk
        bounds = np.searchsorted(key, np.arange(ncell + 1))
        # destination slot for each edge: cell tile base * P + rank within cell
        ranks = np.arange(len(il)) - bounds[key]
        slots = cell_t0[key] * P + ranks
        idx16 = np.zeros(NT * P, np.int16)
        jcol = np.zeros(NT * P, np.float32)
        wpad = np.zeros(NT * P, np.float32)
        idx16[slots] = il
        jcol[slots] = pcol
        wpad[slots] = wc
        per_core.append((idx16.reshape(NT, P), jcol.reshape(NT, P),
                         wpad.reshape(NT, P)))

    batches = []           # (chunk, t0, T)
    t0 = 0
    for ch in range(geo.NCHUNK):
        tc_ = int(ntiles[ch].sum())
        off = 0
        while off < tc_:
            T = min(TBATCH, tc_ - off)
            batches.append((ch, t0 + off, T))
            off += T
        t0 += tc_

    NB = len(batches)
    out_cores = []
    for c in range(NCORE):
        idx16, jcol, wpad = per_core[c]
        idxw = np.zeros((NB, P, TBATCH * P // 16), np.int16)
        for bi, (ch, t0_, T) in enumerate(batches):
            flat = idx16[t0_:t0_ + T].reshape(-1)
            wr = flat.reshape(-1, 16).T                    # [16, T*8]
            idxw[bi, :, :wr.shape[1]] = np.tile(wr, (8, 1))
        jb = np.ascontiguousarray(jcol.T).astype(ml_dtypes.bfloat16)
        wf = np.ascontiguousarray(wpad.T).astype(np.float32)
        out_cores.append({"idx": idxw, "jb": jb, "wf": wf})

    tile_blk = np.zeros(NT, np.int64)
    first = np.zeros(NT, bool)
    last = np.zeros(NT, bool)
    t = 0
    for ch in range(geo.NCHUNK):
        for b in range(NBLK):
            nt = int(ntiles[ch, b])
            tile_blk[t:t + nt] = b
            first[t] = True
            last[t + nt - 1] = True
            t += nt
    meta = {"ntiles": ntiles, "NT": NT, "batches": batches,
            "tile_blk": tile_blk, "first": first, "last": last}
    return meta, out_cores


# ---------------------------------------------------------------- builder
def _build(geo, meta_xy, meta_yz):
    NBLK, K, YC = geo.NBLK, geo.K, geo.YC
    KB = NBLK * K
    nc = bacc.Bacc(get_trn_type() or "TRN2", target_bir_lowering=False,
                   debug=False, num_swdge_queues=NQUEUE)
    # activation() float bias/scale values must exist as const APs
    for v in (EPS, BETA):
        t = nc.alloc_sbuf_tensor(f"const-float32-{v}", [P, 1], F32)
        nc.gpsimd.memset(t.ap(), v)
        nc.const_aps.aps[(F32, v)] = t.ap()
    nc.all_engine_barrier()

    sxl = nc.dram_tensor("sxl", [geo.NUM, K], F32, kind="ExternalInput")
    szl = nc.dram_tensor("szl", [geo.NUM, K], F32, kind="ExternalInput")
    sxs = nc.dram_tensor("sxs", [YC, K], F32, kind="ExternalInput")
    sys_ = nc.dram_tensor("sys", [YC, K], F32, kind="ExternalInput")
    szs = nc.dram_tensor("szs", [YC, K], F32, kind="ExternalInput")
    ident_in = nc.dram_tensor("ident", [P, P], F32, kind="ExternalInput")
    iota_in = nc.dram_tensor("iota", [P, P], BF16, kind="ExternalInput")
    ones_in = nc.dram_tensor("ones", [P, P + 1], F32, kind="ExternalInput")
    vmask_in = nc.dram_tensor("vmask", [P, NBLK], F32, kind="ExternalInput")

    ins = {}
    for s, meta in (("xy", meta_xy), ("yz", meta_yz)):
        NB = len(meta["batches"])
        NT = meta["NT"]
        ins[s] = {
            "idx": nc.dram_tensor(f"idx_{s}", [NB, P, TBATCH * 8], I16,
                                  kind="ExternalInput"),
            "jb": nc.dram_tensor(f"jb_{s}", [P, NT], BF16, kind="ExternalInput"),
            "wf": nc.dram_tensor(f"wf_{s}", [P, NT], F32, kind="ExternalInput"),
        }
    loss_out = nc.dram_tensor("loss", [1, 1], F32, kind="ExternalOutput")

    ARS = 2 * K * K + 256
    ar1_in = nc.dram_tensor("ar1_in", [ARS], F32)
    ar1_out = nc.dram_tensor("ar1_out", [ARS], F32, addr_space="Shared")
    ar2_in = nc.dram_tensor("ar2_in", [64], F32)
    ar2_out = nc.dram_tensor("ar2_out", [64], F32, addr_space="Shared")
    groups = [list(range(NCORE))]

    with tile.TileContext(nc) as tc, ExitStack() as es:
        pp = es.enter_context(tc.tile_pool(name="persist", bufs=1))
        sp = es.enter_context(tc.tile_pool(name="small", bufs=1))
        stp = es.enter_context(tc.tile_pool(name="statps", bufs=1, space="PSUM"))

        ident = pp.tile([P, P], F32)
        nc.sync.dma_start(ident[:], ident_in[:])
        iota = pp.tile([P, P], BF16)
        nc.sync.dma_start(iota[:], iota_in[:])
        ones = pp.tile([P, P + 1], F32)
        nc.sync.dma_start(ones[:], ones_in[:])
        vmask = pp.tile([P, NBLK], F32)
        nc.sync.dma_start(vmask[:], vmask_in[:])

        A_sb = pp.tile([P, NBLK * (K + 1)], F32)
        C_sb = pp.tile([P, NBLK * (K + 1)], F32)
        SY_sb = pp.tile([P, KB], F32)

        stats = stp.tile([1, 256], F32)

        # ---------------- phase 0: shard softmax + stats --------------------
        with tc.tile_pool(name="shard", bufs=1) as shp:
            def shard_stats(src, col_off, ent_col, sy_dst):
                sh = shp.tile([P, KB], F32, tag="shard")
                full_p = YC // NBLK
                nfull = full_p * NBLK
                rem = YC - nfull
                if rem or full_p + 1 < P:
                    nc.vector.memset(sh[:], 0.0)
                nc.sync.dma_start(
                    sh[:full_p, :],
                    src[0:nfull].rearrange("(p b) k -> p (b k)", b=NBLK))
                if rem:
                    nc.sync.dma_start(
                        sh[full_p:full_p + 1, 0:rem * K],
                        src[nfull:YC].rearrange("r k -> (r k)")[None, :])
                nc.scalar.activation(out=sh[:], in_=sh[:], func=AF.Exp)
                z = sp.tile([P, NBLK], F32, tag="z0")
                nc.vector.reduce_sum(
                    z[:], sh[:].rearrange("p (b k) -> p b k", k=K), axis=AX)
                nc.vector.reciprocal(z[:], z[:])
                # zero out invalid (p, b) cells via the validity mask
                nc.vector.tensor_tensor(out=z[:], in0=z[:], in1=vmask[:],
                                        op=OP.mult)
                dst = sy_dst if sy_dst is not None else sh
                nc.vector.tensor_tensor(
                    out=dst[:].rearrange("p (b k) -> p b k", k=K),
                    in0=sh[:].rearrange("p (b k) -> p b k", k=K),
                    in1=z[:, :, None].to_broadcast([P, NBLK, K]),
                    op=OP.mult)
                t1 = sp.tile([P, K], F32, tag="t1c")
                nc.vector.reduce_sum(
                    t1[:], dst[:].rearrange("p (b k) -> p k b", k=K), axis=AX)
                nc.tensor.matmul(out=stats[:, col_off:col_off + K],
                                 lhsT=ones[:, 0:1], rhs=t1[:],
                                 start=True, stop=True)
                ln = shp.tile([P, KB], F32, tag="lnsh")
                nc.scalar.activation(out=ln[:], in_=dst[:], func=AF.Ln, bias=EPS)
                nc.vector.tensor_tensor(out=ln[:], in0=ln[:], in1=dst[:],
                                        op=OP.mult)
                er = sp.tile([P, 1], F32, tag="entr")
                nc.vector.reduce_sum(er[:], ln[:], axis=AX)
                nc.tensor.matmul(out=stats[:, ent_col:ent_col + 1],
                                 lhsT=ones[:, 0:1], rhs=er[:],
                                 start=True, stop=True)

            shard_stats(sxs, 0, 192, None)
            shard_stats(sys_, 64, 193, SY_sb)
            shard_stats(szs, 128, 194, None)

        # ---------------- phase 1: edge segment sums ------------------------
        with (
            tc.tile_pool(name="gath", bufs=2) as gp,
            tc.tile_pool(name="oneh", bufs=2) as op_,
            tc.tile_pool(name="gext", bufs=2) as gxp,
            tc.tile_pool(name="idxp", bufs=3) as ixp,
            tc.tile_pool(name="perb", bufs=3) as pbp,
            tc.tile_pool(name="setc", bufs=1) as scp,
            tc.tile_pool(name="apsum", bufs=3, space="PSUM") as apsp,
        ):
            def process_set(tab, io, meta, acc):
                NT = meta["NT"]
                jb = scp.tile([P, NT], BF16, tag="jbt")
                nc.sync.dma_start(jb[:], io["jb"][:])
                wf = scp.tile([P, NT], F32, tag="wft")
                nc.sync.dma_start(wf[:], io["wf"][:])
                tile_blk = meta["tile_blk"]
                first, last = meta["first"], meta["last"]
                qrr = [0]
                cur = {"psg": None, "grp": -1, "ch": -1}

                def close_grp():
                    g, ch = cur["grp"], cur["ch"]
                    lo = g * GRP * (K + 1)
                    width = min(GRP, NBLK - g * GRP) * (K + 1)
                    if ch == 0:
                        nc.vector.tensor_copy(
                            out=acc[:, lo:lo + width],
                            in_=cur["psg"][:, 0:width])
                    else:
                        nc.vector.tensor_tensor(
                            out=acc[:, lo:lo + width],
                            in0=acc[:, lo:lo + width],
                            in1=cur["psg"][:, 0:width], op=OP.add)
                    cur["psg"] = None

                for bi, (ch, t0, T) in enumerate(meta["batches"]):
                    it = ixp.tile([P, TBATCH * 8], I16, tag="idxt")
                    nc.sync.dma_start(it[:], io["idx"][bi])
                    gt = gp.tile([P, TBATCH * K], F32, tag="gt")
                    nidx = T * P
                    nc.gpsimd.dma_gather(
                        gt[:].rearrange("p (t k) -> p t k", k=K)[:, 0:T, :],
                        tab[ch * geo.CHUNK:(ch + 1) * geo.CHUNK, :],
                        it[:, 0:nidx // 16], nidx, nidx, K,
                        single_packet=False, queue_num=qrr[0])
                    qrr[0] = (qrr[0] + 1) % NQUEUE
                    et = gxp.tile([P, TBATCH * K], BF16, tag="et")
                    nc.scalar.activation(out=et[:, 0:T * K], in_=gt[:, 0:T * K],
                                         func=AF.Exp)
                    z = pbp.tile([P, TBATCH], F32, tag="zb")
                    nc.vector.reduce_sum(
                        z[:, 0:T],
                        et[:].rearrange("p (t k) -> p t k", k=K)[:, 0:T, :],
                        axis=AX)
                    nc.vector.reciprocal(z[:, 0:T], z[:, 0:T])
                    ct = pbp.tile([P, TBATCH], BF16, tag="cb")
                    nc.vector.tensor_tensor(out=ct[:, 0:T], in0=z[:, 0:T],
                                            in1=wf[:, t0:t0 + T], op=OP.mult)
                    gx = gxp.tile([P, TBATCH * (K + 1)], BF16, tag="gx")
                    gxv = gx[:].rearrange("p (t k) -> p t k", k=K + 1)
                    nc.vector.tensor_tensor(
                        out=gxv[:, 0:T, 0:K],
                        in0=et[:].rearrange("p (t k) -> p t k", k=K)[:, 0:T, :],
                        in1=ct[:, 0:T, None].to_broadcast([P, T, K]),
                        op=OP.mult)
                    nc.vector.tensor_copy(out=gxv[:, 0:T, K:K + 1],
                                          in_=wf[:, t0:t0 + T, None])
                    ob = op_.tile([P, TBATCH * P], BF16, tag="ob")
                    nc.vector.tensor_tensor(
                        out=ob[:].rearrange("p (t q) -> p t q", q=P)[:, 0:T, :],
                        in0=iota[:, None, :].to_broadcast([P, T, P]),
                        in1=jb[:, t0:t0 + T, None].to_broadcast([P, T, P]),
                        op=OP.is_equal)
                    for t in range(T):
                        tg = t0 + t
                        b = int(tile_blk[tg])
                        g = b // GRP
                        if g != cur["grp"] or ch != cur["ch"]:
                            if cur["psg"] is not None:
                                close_grp()
                            cur["psg"] = apsp.tile([P, GRP * (K + 1)], F32,
                                                   tag="apsg", name="apsg")
                            cur["grp"], cur["ch"] = g, ch
                        lo = (b % GRP) * (K + 1)
                        nc.tensor.matmul(
                            out=cur["psg"][:, lo:lo + K + 1],
                            lhsT=ob[:, t * P:(t + 1) * P],
                            rhs=gx[:, t * (K + 1):(t + 1) * (K + 1)],
                            start=bool(first[tg]), stop=bool(last[tg]))
                if cur["psg"] is not None:
                    close_grp()

            process_set(sxl, ins["xy"], meta_xy, A_sb)
            process_set(szl, ins["yz"], meta_yz, C_sb)

        # ---------------- phase 2a: omega, Mnorm, E partials ----------------
        dX = A_sb[:].rearrange("p (b k) -> p b k", k=K + 1)[:, :, K]
        dZ = C_sb[:].rearrange("p (b k) -> p b k", k=K + 1)[:, :, K]
        prod = sp.tile([P, NBLK], F32, tag="prod")
        nc.vector.tensor_tensor(out=prod[:], in0=dX, in1=dZ, op=OP.mult)
        valid = sp.tile([P, NBLK], F32, tag="valid")
        nc.vector.tensor_scalar(out=valid[:], in0=prod[:], scalar1=0.0,
                                scalar2=None, op0=OP.not_equal)
        omega = sp.tile([P, NBLK], F32, tag="omega")
        nc.vector.tensor_scalar(out=omega[:], in0=prod[:], scalar1=EPS,
                                scalar2=None, op0=OP.add)
        nc.vector.reciprocal(omega[:], omega[:])
        nc.vector.tensor_tensor(out=omega[:], in0=omega[:], in1=valid[:],
                                op=OP.mult)
        mn = sp.tile([P, NBLK], F32, tag="mn")
        nc.vector.tensor_tensor(out=mn[:], in0=prod[:], in1=valid[:], op=OP.mult)
        mnr = sp.tile([P, 1], F32, tag="mnr")
        nc.vector.reduce_sum(mnr[:], mn[:], axis=AX)
        nc.tensor.matmul(out=stats[:, 195:196], lhsT=ones[:, 0:1],
                         rhs=mnr[:], start=True, stop=True)

        bp = es.enter_context(tc.tile_pool(name="blk", bufs=4))
        with tc.tile_pool(name="epsum", bufs=1, space="PSUM") as eps_:
            exy_ps = eps_.tile([K, K], F32, tag="exy")
            eyz_ps = eps_.tile([K, K], F32, tag="eyz")
            for b in range(NBLK):
                ay = bp.tile([P, K], F32, tag="ay")
                nc.vector.tensor_scalar(
                    out=ay[:], in0=A_sb[:, b * (K + 1):b * (K + 1) + K],
                    scalar1=omega[:, b:b + 1], scalar2=None, op0=OP.mult)
                cy = bp.tile([P, K], F32, tag="cy")
                nc.vector.tensor_scalar(
                    out=cy[:], in0=C_sb[:, b * (K + 1):b * (K + 1) + K],
                    scalar1=omega[:, b:b + 1], scalar2=None, op0=OP.mult)
                nc.tensor.matmul(out=exy_ps[:], lhsT=ay[:],
                                 rhs=SY_sb[:, b * K:(b + 1) * K],
                                 start=(b == 0), stop=(b == NBLK - 1))
                nc.tensor.matmul(out=eyz_ps[:], lhsT=SY_sb[:, b * K:(b + 1) * K],
                                 rhs=cy[:], start=(b == 0), stop=(b == NBLK - 1))
            exy_sb = sp.tile([K, K], F32, tag="exysb")
            nc.vector.tensor_copy(out=exy_sb[:], in_=exy_ps[:])
            eyz_sb = sp.tile([K, K], F32, tag="eyzsb")
            nc.vector.tensor_copy(out=eyz_sb[:], in_=eyz_ps[:])
        stats_sb = sp.tile([1, 256], F32, tag="statsb")
        nc.vector.memset(stats_sb[:], 0.0)
        nc.vector.tensor_copy(out=stats_sb[:, 0:196], in_=stats[:, 0:196])

        nc.sync.dma_start(ar1_in[0:K * K], exy_sb[:])
        nc.sync.dma_start(ar1_in[K * K:2 * K * K], eyz_sb[:])
        nc.sync.dma_start(ar1_in[2 * K * K:ARS], stats_sb[:])
        nc.gpsimd.collective_compute(
            "AllReduce", OP.add, replica_groups=groups,
            ins=[ar1_in[:]], outs=[ar1_out[:]])
        exy = sp.tile([K, K], F32, tag="exyg")
        nc.sync.dma_start(exy[:], ar1_out[0:K * K].rearrange("(a b) -> a b", b=K))
        eyz = sp.tile([K, K], F32, tag="eyzg")
        nc.sync.dma_start(eyz[:],
                          ar1_out[K * K:2 * K * K].rearrange("(a b) -> a b", b=K))
        stg = sp.tile([1, 256], F32, tag="stg")
        nc.sync.dma_start(stg[:], ar1_out[2 * K * K:ARS][None, :])

        # ---------------- phase 2b: alpha/gamma + Q_exp (replicated) --------
        with tc.tile_pool(name="p2psum", bufs=2, space="PSUM") as ps2:
            def pe_t(src, rows, cols, tag):
                pt = ps2.tile([P, P], F32, tag="scr2b", name="pt")
                nc.tensor.transpose(out=pt[0:cols, 0:rows], in_=src,
                                    identity=ident[0:rows, 0:rows])
                out = sp.tile([cols, rows], F32, tag=tag)
                nc.vector.tensor_copy(out=out[:], in_=pt[0:cols, 0:rows])
                return out

            invmn = sp.tile([P, 1], F32, tag="invmn")
            bps = ps2.tile([P, 1], F32, tag="scr2b", name="bps")
            nc.tensor.matmul(out=bps[:], lhsT=ones[0:1, 1:P + 1],
                             rhs=stg[:, 195:196], start=True, stop=True)
            nc.vector.tensor_scalar(out=invmn[:], in0=bps[:], scalar1=EPS,
                                    scalar2=None, op0=OP.add)
            nc.vector.reciprocal(invmn[:], invmn[:])

            exn = sp.tile([K, K], F32, tag="exn")
            nc.vector.tensor_scalar(out=exn[:], in0=exy[:],
                                    scalar1=invmn[0:K, :], scalar2=None,
                                    op0=OP.mult)
            ezn = sp.tile([K, K], F32, tag="ezn")
            nc.vector.tensor_scalar(out=ezn[:], in0=eyz[:],
                                    scalar1=invmn[0:K, :], scalar2=None,
                                    op0=OP.mult)

            def softmax_rows(src, tag):
                e = sp.tile([K, K], F32, tag=tag + "e")
                nc.scalar.activation(out=e[:], in_=src[:], func=AF.Exp,
                                     scale=BETA)
                zz = sp.tile([K, 1], F32, tag=tag + "z")
                nc.vector.reduce_sum(zz[:], e[:], axis=AX)
                nc.vector.reciprocal(zz[:], zz[:])
                nc.vector.tensor_scalar(out=e[:], in0=e[:], scalar1=zz[:],
                                        scalar2=None, op0=OP.mult)
                return e

            exnT = pe_t(exn[:], K, K, "exnT")
            alphaT = softmax_rows(exnT, "aT")          # [m, l]
            alpha = pe_t(alphaT[:], K, K, "alpha")     # [l, m]
            gamma = softmax_rows(ezn, "gm")            # [m, n]
            gammaT = pe_t(gamma[:], K, K, "gmT")       # [n, m]

            aX = sp.tile([K, 1], F32, tag="aX")
            nc.vector.reduce_sum(aX[:], exn[:], axis=AX)
            aY = sp.tile([K, 1], F32, tag="aY")
            nc.vector.reduce_sum(aY[:], exnT[:], axis=AX)
            eznT = pe_t(ezn[:], K, K, "eznT")
            aZ = sp.tile([K, 1], F32, tag="aZ")
            nc.vector.reduce_sum(aZ[:], eznT[:], axis=AX)

            tl_ps = ps2.tile([K, 1], F32, tag="scr2b", name="tl_ps")
            nc.tensor.matmul(out=tl_ps[:], lhsT=alpha[:], rhs=aX[:],
                             start=True, stop=True)
            tr_ps = ps2.tile([K, 1], F32, tag="scr2b", name="tr_ps")
            nc.tensor.matmul(out=tr_ps[:], lhsT=gammaT[:], rhs=aZ[:],
                             start=True, stop=True)
            qe = sp.tile([K, 1], F32, tag="qe")
            nc.vector.tensor_tensor(out=qe[:], in0=tl_ps[:], in1=aY[:],
                                    op=OP.mult)
            nc.vector.tensor_tensor(out=qe[:], in0=qe[:], in1=tr_ps[:],
                                    op=OP.mult)
            qexp_ps = ps2.tile([1, 1], F32, tag="scr2b", name="qexp_ps")
            nc.tensor.matmul(out=qexp_ps[:], lhsT=ones[0:K, 0:1], rhs=qe[:],
                             start=True, stop=True)
            qexp = sp.tile([1, 1], F32, tag="qexp")
            nc.vector.tensor_copy(out=qexp[:], in_=qexp_ps[:])

            # ---------------- phase 2c: Q_obs partial -----------------------
            qacc = sp.tile([P, K], F32, tag="qacc")
            nc.vector.memset(qacc[:], 0.0)
            for b in range(NBLK):
                ay = bp.tile([P, K], F32, tag="ay2")
                nc.vector.tensor_scalar(
                    out=ay[:], in0=A_sb[:, b * (K + 1):b * (K + 1) + K],
                    scalar1=omega[:, b:b + 1], scalar2=None, op0=OP.mult)
                cy = bp.tile([P, K], F32, tag="cy2")
                nc.vector.tensor_scalar(
                    out=cy[:], in0=C_sb[:, b * (K + 1):b * (K + 1) + K],
                    scalar1=omega[:, b:b + 1], scalar2=None, op0=OP.mult)
                ayT_ps = ps2.tile([K, P], F32, tag="ayT", bufs=1)
                nc.tensor.transpose(out=ayT_ps[:], in_=ay[:], identity=ident[:])
                ayT = bp.tile([K, P], F32, tag="ayTs")
                nc.vector.tensor_copy(out=ayT[:], in_=ayT_ps[:])
                cyT_ps = ps2.tile([K, P], F32, tag="cyT", bufs=1)
                nc.tensor.transpose(out=cyT_ps[:], in_=cy[:], identity=ident[:])
                cyT = bp.tile([K, P], F32, tag="cyTs")
                nc.vector.tensor_copy(out=cyT[:], in_=cyT_ps[:])
                l_ps = ps2.tile([P, K], F32, tag="lps", bufs=1)
                nc.tensor.matmul(out=l_ps[:], lhsT=ayT[:], rhs=alpha[:],
                                 start=True, stop=True)
                r_ps = ps2.tile([P, K], F32, tag="rps", bufs=1)
                nc.tensor.matmul(out=r_ps[:], lhsT=cyT[:], rhs=gammaT[:],
                                 start=True, stop=True)
                t1 = bp.tile([P, K], F32, tag="t1q")
                nc.vector.tensor_tensor(out=t1[:], in0=l_ps[:],
                                        in1=SY_sb[:, b * K:(b + 1) * K],
                                        op=OP.mult)
                nc.vector.tensor_tensor(out=t1[:], in0=t1[:], in1=r_ps[:],
                                        op=OP.mult)
                nc.vector.tensor_tensor(out=qacc[:], in0=qacc[:], in1=t1[:],
                                        op=OP.add)
            qr = sp.tile([P, 1], F32, tag="qr")
            nc.vector.reduce_sum(qr[:], qacc[:], axis=AX)
            qo_ps = ps2.tile([1, 1], F32, tag="scr2b", name="qo_ps")
            nc.tensor.matmul(out=qo_ps[:], lhsT=ones[:, 0:1], rhs=qr[:],
                             start=True, stop=True)
            qob = sp.tile([1, 64], F32, tag="qob")
            nc.vector.memset(qob[:], 0.0)
            nc.vector.tensor_copy(out=qob[:, 0:1], in_=qo_ps[:])
            nc.sync.dma_start(ar2_in[:], qob[:])
            nc.gpsimd.collective_compute(
                "AllReduce", OP.add, replica_groups=groups,
                ins=[ar2_in[:]], outs=[ar2_out[:]])
            qot = sp.tile([1, 1], F32, tag="qot")
            nc.sync.dma_start(qot[:], ar2_out[0:1][None, :])

            # ---------------- loss assembly ---------------------------------
            sc = sp.tile([1, 8], F32, tag="scr")
            nc.vector.tensor_scalar(out=sc[:, 0:1], in0=qot[:],
                                    scalar1=invmn[0:1, :], scalar2=None,
                                    op0=OP.mult)
            nc.vector.tensor_scalar(out=sc[:, 1:2], in0=qexp[:],
                                    scalar1=invmn[0:1, :], scalar2=None,
                                    op0=OP.mult)
            nc.vector.tensor_scalar(out=sc[:, 1:2], in0=sc[:, 1:2],
                                    scalar1=GAMMA_, scalar2=None, op0=OP.mult)
            nc.vector.tensor_tensor(out=sc[:, 2:3], in0=sc[:, 1:2],
                                    in1=sc[:, 0:1], op=OP.subtract)
            for mi in range(3):
                cs = stg[:, 64 * mi:64 * mi + K]
                sq = sp.tile([1, K], F32, tag="sqc")
                nc.vector.tensor_tensor(out=sq[:], in0=cs, in1=cs, op=OP.mult)
                fr = sp.tile([1, 1], F32, tag="frc")
                nc.vector.reduce_sum(fr[:], sq[:], axis=AX)
                nc.scalar.activation(out=fr[:], in_=fr[:], func=AF.Sqrt)
                nc.vector.tensor_scalar(
                    out=fr[:], in0=fr[:],
                    scalar1=float(LAM * np.sqrt(float(K)) / geo.NUM),
                    scalar2=float(-LAM), op0=OP.mult, op1=OP.add)
                nc.vector.tensor_tensor(out=sc[:, 2:3], in0=sc[:, 2:3],
                                        in1=fr[:], op=OP.add)
            er = sp.tile([1, 1], F32, tag="ereg")
            nc.vector.reduce_sum(er[:], stg[:, 192:195], axis=AX)
            nc.vector.tensor_scalar(out=er[:], in0=er[:],
                                    scalar1=float(ENTW / geo.NUM), scalar2=None,
                                    op0=OP.mult)
            nc.vector.tensor_tensor(out=sc[:, 2:3], in0=sc[:, 2:3],
                                    in1=er[:], op=OP.add)
            nc.sync.dma_start(loss_out[:], sc[:, 2:3])
    nc.compile()
    return nc


# ---------------------------------------------------------------- entry
def _make_in_maps(geo, Sx, Sy, Sz, cores_xy, cores_yz):
    ident = np.eye(P, dtype=np.float32)
    iota = np.tile(np.arange(P, dtype=np.float32), (P, 1)).astype(ml_dtypes.bfloat16)
    ones = np.ones((P, P + 1), np.float32)
    pb = np.arange(P)[:, None] * geo.NBLK + np.arange(geo.NBLK)[None, :]
    vmask_np = (pb < geo.YC).astype(np.float32)
    YC = geo.YC
    in_maps = []
    for c in range(NCORE):
        in_maps.append({
            "sxl": Sx, "szl": Sz,
            "sxs": np.ascontiguousarray(Sx[c * YC:(c + 1) * YC]),
            "sys": np.ascontiguousarray(Sy[c * YC:(c + 1) * YC]),
            "szs": np.ascontiguousarray(Sz[c * YC:(c + 1) * YC]),
            "ident": ident, "iota": iota, "ones": ones, "vmask": vmask_np,
            "idx_xy": cores_xy[c]["idx"], "jb_xy": cores_xy[c]["jb"],
            "wf_xy": cores_xy[c]["wf"],
            "idx_yz": cores_yz[c]["idx"], "jb_yz": cores_yz[c]["jb"],
            "wf_yz": cores_yz[c]["wf"],
        })
    return in_maps


def kernel(Sx_logits, Sy_logits, Sz_logits, edge_index_XY, edge_weight_XY,
           edge_index_YZ, edge_weight_YZ, _geo=None, _sim=False):
    geo = _geo or GEO
    Sx = np.ascontiguousarray(np.asarray(Sx_logits, np.float32))
    Sy = np.ascontiguousarray(np.asarray(Sy_logits, np.float32))
    Sz = np.ascontiguousarray(np.asarray(Sz_logits, np.float32))

    meta_xy, cores_xy = _prep_edges(geo, np.asarray(edge_index_XY),
                                    np.asarray(edge_weight_XY))
    meta_yz, cores_yz = _prep_edges(geo, np.asarray(edge_index_YZ),
                                    np.asarray(edge_weight_YZ))
    nc = _build(geo, meta_xy, meta_yz)
    in_maps = _make_in_maps(geo, Sx, Sy, Sz, cores_xy, cores_yz)

    if _sim:
        import concourse.bass_interp as bass_interp
        sim = bass_interp.MultiCoreSim(nc, NCORE)
        for c in range(NCORE):
            for k, v in in_maps[c].items():
                sim.cores[c].tensor(k)[:] = np.asarray(v)
        sim.simulate()
        return np.float32(np.asarray(sim.cores[0].tensor("loss"))[0, 0])

    res = run_bass_kernel_spmd(nc, in_maps, list(range(NCORE)),
                               trace=_TRACE[0])
    if _TRACE[0]:
        kernel.last_exec_ns = res.exec_time_ns
    return np.float32(res.results[0]["loss"][0, 0])
